# revision 8
# baseline (speedup 1.0000x reference)
# Trainium2 Bass kernel for LinearAttention (nn_LinearAttention_87686052315975).
#
# Reference computation (per batch element b of 16):
#   xf = x[b].reshape(512, 4096)                      # [c, l]
#   qkv = w_qkv @ xf                                  # [1536, l]
#   q, k, v split into 8 heads x 64 dims
#   k = softmax(k, axis=l)
#   context_h = k_h @ v_h^T                           # [64, 64]
#   out_h = context_h^T @ q_h                         # [64, l]
#   y = w_out @ concat(out_h) + b_out                 # [512, l]
#
# Sharding: data-parallel over batch. 16 batches / 8 cores = 2 per core.
# No collectives needed; each core produces its own output slice.
#
# Per-core kernel structure (per batch, l chunked by 512):
#   Pass A: q = w_q^T-form matmul (kept resident in SBUF, [512, 4096]);
#           kT/vT computed transposed (l on partitions) so the context
#           contraction over l maps onto the PE K dim;
#           E = exp(kT) (no max subtraction needed: |k| ~ N(0,1));
#           ctx_h[d, e] += E_h^T-contract-vT_h via matmul, with a ones
#           column appended to vT so column 64 accumulates rowsum(E).
#   Finalize: ctx_n = ctx * (1/s) per row; pack head pairs into a
#           block-diagonal [128, 128] lhsT via SBUF->SBUF DMA.
#   Pass B: out = ctxP^T-contract-q (one matmul per head pair);
#           y = w_out^T-form matmul + bias; DMA out.
#
# All big matmuls run as float32r (split-precision fp32, 1 cycle/row at
# N>=256 vs 4 for plain fp32). The small context matmuls (N=65) run at
# 4 cycles/row regardless; dtype for them is configurable.

import os
import numpy as np
from contextlib import ExitStack

import concourse.bass as bass
import concourse.bacc as bacc
import concourse.mybir as mybir
import concourse.tile as tile

# ---- problem constants (hardcoded per contract) ----
B, DIM, HGT, WID = 16, 512, 64, 64
L = HGT * WID            # 4096
HEADS, DH = 8, 64
HIDDEN = HEADS * DH      # 512
NCORES = 8
BPC = B // NCORES        # 2 batches per core
P = 128
CHUNK = 512
NCHUNK = L // CHUNK      # 8
KT = DIM // P            # 4 contraction tiles over channels
MT = DIM // P            # 4 output row tiles
LM = CHUNK // P          # 4 l-subtiles per chunk
NPAIR = HEADS // 2       # 4 head pairs
VW = DH + 2              # per-head vT width: 64 v cols + 2 ones cols (even N)

F32 = mybir.dt.float32
F32R = mybir.dt.float32r
MM_DT = mybir.dt.float32r     # dtype for the big (N=512) matmuls
CTX_DT = mybir.dt.float32r    # dtype for the small context matmuls


def _mm(ap, dt):
    return ap.bitcast(dt)


def build_kernel(ctx: ExitStack, tc: "tile.TileContext", x_in, wqkvT_in, woutT_in,
                 bias_in, y_out):
    nc = tc.nc

    wpool = ctx.enter_context(tc.tile_pool(name="weights", bufs=1))
    qpool = ctx.enter_context(tc.tile_pool(name="qres", bufs=1))
    xpool = ctx.enter_context(tc.tile_pool(name="xc", bufs=8))
    epool = ctx.enter_context(tc.tile_pool(name="ev", bufs=8))
    opool = ctx.enter_context(tc.tile_pool(name="osb", bufs=8))
    ypool = ctx.enter_context(tc.tile_pool(name="ysb", bufs=3))
    cpool = ctx.enter_context(tc.tile_pool(name="ctxacc", bufs=1))
    npool = ctx.enter_context(tc.tile_pool(name="nrm", bufs=4))
    ppool = ctx.enter_context(tc.tile_pool(name="ctxp", bufs=2))
    psmm = ctx.enter_context(tc.tile_pool(name="psmm", bufs=4, space="PSUM"))
    psctx = ctx.enter_context(tc.tile_pool(name="psctx", bufs=2, space="PSUM"))

    # ---- load weights once ----
    wqkv_sb = []
    for k in range(KT):
        t = wpool.tile([P, 3 * HIDDEN], F32R, tag=f"wqkv{k}", name=f"wqkv{k}")
        nc.sync.dma_start(t[:], wqkvT_in[k * P:(k + 1) * P, :])
        wqkv_sb.append(t)
    wout_sb = []
    for k in range(KT):
        t = wpool.tile([P, DIM], F32R, tag=f"wout{k}", name=f"wout{k}")
        nc.sync.dma_start(t[:], woutT_in[k * P:(k + 1) * P, :])
        wout_sb.append(t)
    bias_sb = wpool.tile([P, MT], F32, tag="bias", name="bias")
    nc.sync.dma_start(bias_sb[:], bias_in[:])

    for b in range(BPC):
        # persistent q for this batch: 4 tiles [128, 4096]
        q_sb = [qpool.tile([P, L], F32R, tag=f"q{m}", name=f"q{m}") for m in range(MT)]
        # per-head context accumulators [64, 65] (col 64 = rowsum of E)
        ctx_acc = [cpool.tile([DH, VW], F32, tag=f"ctxacc{h}", name=f"ctxacc{h}")
                   for h in range(HEADS)]

        # ---------------- Pass A ----------------
        for i in range(NCHUNK):
            ls = slice(i * CHUNK, (i + 1) * CHUNK)
            xc = []
            for k in range(KT):
                t = xpool.tile([P, CHUNK], F32R, tag="xc", name="xc")
                nc.sync.dma_start(t[:], x_in[b, k * P:(k + 1) * P, ls])
                xc.append(t)

            # q projection: q[o, l] for o-tile m
            for m in range(MT):
                ps = psmm.tile([P, CHUNK], F32, tag="mm", name="mm")
                for k in range(KT):
                    nc.tensor.matmul(
                        ps[:],
                        _mm(wqkv_sb[k][:, m * P:(m + 1) * P], MM_DT),
                        _mm(xc[k][:], MM_DT),
                        start=(k == 0), stop=(k == KT - 1))
                nc.vector.tensor_copy(q_sb[m][:, ls], ps[:])

            # kT/vT projection (l on partitions), exp, ones-append
            E_t, vT_t = [], []
            for lm in range(LM):
                # k half -> E = exp(kT)
                ps = psmm.tile([P, CHUNK], F32, tag="mm", name="mm")
                for k in range(KT):
                    nc.tensor.matmul(
                        ps[:],
                        _mm(xc[k][:, lm * P:(lm + 1) * P], MM_DT),
                        _mm(wqkv_sb[k][:, HIDDEN:2 * HIDDEN], MM_DT),
                        start=(k == 0), stop=(k == KT - 1))
                e = epool.tile([P, CHUNK], F32R, tag="E", name="E")
                nc.scalar.activation(e[:], ps[:],
                                     mybir.ActivationFunctionType.Exp)
                E_t.append(e)

                # v half -> vT with a ones column per head ([128, 8*65])
                ps = psmm.tile([P, CHUNK], F32, tag="mm", name="mm")
                for k in range(KT):
                    nc.tensor.matmul(
                        ps[:],
                        _mm(xc[k][:, lm * P:(lm + 1) * P], MM_DT),
                        _mm(wqkv_sb[k][:, 2 * HIDDEN:3 * HIDDEN], MM_DT),
                        start=(k == 0), stop=(k == KT - 1))
                v = epool.tile([P, HEADS * VW], F32R, tag="vT", name="vT")
                v_view = v[:].rearrange("p (h e) -> p h e", e=VW)
                nc.vector.tensor_copy(
                    v_view[:, :, 0:DH],
                    ps[:].rearrange("p (h e) -> p h e", e=DH))
                nc.vector.tensor_scalar(
                    v_view[:, :, DH:DH + 2],
                    ps[:].rearrange("p (h e) -> p h e", e=DH)[:, :, 0:2],
                    0.0, 1.0, mybir.AluOpType.mult, mybir.AluOpType.add)
                vT_t.append(v)

            # context accumulation per head
            for h in range(HEADS):
                pc = psctx.tile([DH, VW], F32, tag="ctx", name="ctx")
                for lm in range(LM):
                    nc.tensor.matmul(
                        pc[:],
                        _mm(E_t[lm][:, h * DH:(h + 1) * DH], CTX_DT),
                        _mm(vT_t[lm][:, h * VW:(h + 1) * VW], CTX_DT),
                        start=(lm == 0), stop=(lm == LM - 1))
                if i == 0:
                    nc.vector.tensor_copy(ctx_acc[h][:], pc[:])
                else:
                    nc.vector.tensor_add(ctx_acc[h][:], ctx_acc[h][:], pc[:])

        # ---------------- Finalize: normalize + block-diag pack ----------
        ctxP = []
        for p in range(NPAIR):
            t = ppool.tile([P, P], F32R, tag=f"p{p}", name=f"p{p}")
            nc.vector.tensor_scalar(t[:], wout_sb[0][:, 0:P], 0.0, None,
                                    mybir.AluOpType.mult)
            ctxP.append(t)
        for h in range(HEADS):
            nc.vector.reciprocal(ctx_acc[h][:, DH:DH + 1],
                                 ctx_acc[h][:, DH:DH + 1])
            nrm = npool.tile([DH, DH], F32R, tag="nrm", name="nrm")
            nc.vector.tensor_scalar_mul(nrm[:], ctx_acc[h][:, 0:DH],
                                        ctx_acc[h][:, DH:DH + 1])
            half = (h % 2) * DH
            nc.sync.dma_start(ctxP[h // 2][half:half + DH, half:half + DH],
                              nrm[:])

        # ---------------- Pass B ----------------
        for i in range(NCHUNK):
            ls = slice(i * CHUNK, (i + 1) * CHUNK)
            out_sb = []
            for p in range(NPAIR):
                ps = psmm.tile([P, CHUNK], F32, tag="mm", name="mm")
                nc.tensor.matmul(ps[:], _mm(ctxP[p][:], MM_DT),
                                 _mm(q_sb[p][:, ls], MM_DT),
                                 start=True, stop=True)
                o = opool.tile([P, CHUNK], F32R, tag="osb", name="osb")
                nc.scalar.copy(o[:], ps[:])
                out_sb.append(o)
            for m in range(MT):
                ps = psmm.tile([P, CHUNK], F32, tag="mm", name="mm")
                for k in range(KT):
                    nc.tensor.matmul(
                        ps[:],
                        _mm(wout_sb[k][:, m * P:(m + 1) * P], MM_DT),
                        _mm(out_sb[k][:], MM_DT),
                        start=(k == 0), stop=(k == KT - 1))
                y = ypool.tile([P, CHUNK], F32, tag="ysb", name="ysb")
                nc.vector.tensor_scalar_add(y[:], ps[:],
                                            bias_sb[:, m:m + 1])
                nc.sync.dma_start(y_out[b, m * P:(m + 1) * P, ls], y[:])


def build_module():
    nc = bacc.Bacc("TRN2", target_bir_lowering=False, debug=False,
                   num_devices=NCORES)
    x_in = nc.dram_tensor("x", [BPC, DIM, L], F32R, kind="ExternalInput")
    wqkvT_in = nc.dram_tensor("w_qkvT", [DIM, 3 * HIDDEN], F32R,
                              kind="ExternalInput")
    woutT_in = nc.dram_tensor("w_outT", [HIDDEN, DIM], F32R,
                              kind="ExternalInput")
    bias_in = nc.dram_tensor("bias", [P, MT], F32, kind="ExternalInput")
    y_out = nc.dram_tensor("y", [BPC, DIM, L], F32, kind="ExternalOutput")
    with tile.TileContext(nc) as tc:
        with ExitStack() as ctx:
            build_kernel(ctx, tc, x_in, wqkvT_in, woutT_in, bias_in, y_out)
    nc.compile()
    return nc


def make_in_maps(x, w_qkv, w_out, b_out):
    x = np.ascontiguousarray(x, dtype=np.float32).reshape(B, DIM, L)
    wqkvT = np.ascontiguousarray(np.asarray(w_qkv, dtype=np.float32).T)
    woutT = np.ascontiguousarray(np.asarray(w_out, dtype=np.float32).T)
    bias = np.ascontiguousarray(
        np.asarray(b_out, dtype=np.float32).reshape(MT, P).T)
    in_maps = []
    for c in range(NCORES):
        in_maps.append({
            "x": x[c * BPC:(c + 1) * BPC],
            "w_qkvT": wqkvT,
            "w_outT": woutT,
            "bias": bias,
        })
    return in_maps


_NC_CACHE = None


def kernel(x, w_qkv, w_out, b_out, *, trace=False, trace_kwargs=None):
    """Full inputs in, full output out. Shards batch across 8 NeuronCores."""
    global _NC_CACHE
    from concourse.bass_utils import run_bass_kernel_spmd

    if _NC_CACHE is None:
        _NC_CACHE = build_module()
    nc = _NC_CACHE

    in_maps = make_in_maps(x, w_qkv, w_out, b_out)
    kw = dict(trace_kwargs or {})
    res = run_bass_kernel_spmd(nc, in_maps, list(range(NCORES)),
                               trace=trace, **kw)
    y = np.empty((B, DIM, HGT, WID), dtype=np.float32)
    for c in range(NCORES):
        y[c * BPC:(c + 1) * BPC] = res.results[c]["y"].reshape(
            BPC, DIM, HGT, WID)
    kernel.last_results = res
    return y
